# revision 26
# baseline (speedup 1.0000x reference)
"""Tensor-parallel attention block for 8 TRN2 NeuronCores.

Reference computation (per nn_Attention_63359357550974):
    qkv = x @ Wqkv + bqkv ; split into q, k_new, v_new per head
    k = concat(k_cache, k_new); v = concat(v_cache, v_new)
    o = softmax(q @ k^T / sqrt(d)) @ v
    out = o @ Wout + bout ; returns (out, k, v)

Sharding: 16 heads / 8 cores = 2 heads per core (tensor parallel).  Each
core computes its heads' QKV columns from the full x, runs attention for
its 4 (batch, head) pairs, and produces a partial output projection
(contracting only its 256 o-columns of Wout).  The host sums the 8
partial projections and adds bout; k/v outputs are assembled host-side
from the cache plus each core's new-key tensors.  No device collectives.

Device layout choices (matmul inputs in fp16: 1 cycle/row PE streaming —
fp32/fp32r stream at 2 cycles/row — while keeping ~1e-3 accuracy):
  - x is host-transposed/cast to xT [dim, tok] fp16 so qkv^T needs no
    on-device transposes; q^T / k_new^T come out directly as [d, tok].
  - scores are computed transposed: S^T[keys, q] = kT-chunk^T @ qT, so
    exp(scale * S^T) -> A^T is directly the PV moving operand.  No
    max-subtraction (|scores*scale| stays < ~6 for this data regime).
  - softmax denominators via ones[128,128]-stationary matmuls: every
    PSUM partition receives the column sum, so the reciprocal runs on
    all 128 lanes and no partition broadcast is needed.
  - o^T = V^T @ A^T accumulated over 16 key chunks, multiplied by the
    replicated reciprocal.
  - partial out[tok, col] = o^T-stationary @ Wout-moving, DMA'd out.
"""

import numpy as np

import concourse.tile as tile
from concourse import bacc, mybir
from concourse.bass_utils import run_bass_kernel_spmd

P = 128
B = 2
L = 1024  # new tokens per batch
LP = 1024  # cached keys per batch
D = 2048
H = 16
DH = 128  # head dim
N_CORES = 8
HPC = H // N_CORES  # heads per core = 2
TOK = B * L  # 2048
DC = D // P  # 16 dim chunks
KC = (LP + L) // P  # 16 key chunks per (b, h)
KCP = LP // P  # 8 cached key chunks
QB = 512  # q block (moving free dim)
NQB = L // QB  # 2

F32 = mybir.dt.float32
F16 = mybir.dt.float16
SCALE = 1.0 / float(np.sqrt(DH))

LAST_RESULT = None
_CACHE = {}


def _build():
    nc = bacc.Bacc(None, target_bir_lowering=False)
    AF = mybir.ActivationFunctionType
    OP = mybir.AluOpType
    with tile.TileContext(nc) as tc:
        with tc.tile_pool(name="dram", bufs=1, space="DRAM") as dram:
            xp_d = dram.tile((P, DC, TOK), F16, kind="ExternalInput")
            wqk_d = dram.tile((P, DC, 4 * P), F16, kind="ExternalInput")
            wv_d = dram.tile((P, DC, HPC * DH), F16, kind="ExternalInput")
            bqk_d = dram.tile((4 * P,), F32, kind="ExternalInput")
            bv_d = dram.tile((HPC * DH,), F32, kind="ExternalInput")
            kcT_d = dram.tile((B, HPC, DH, LP), F16, kind="ExternalInput")
            vc_d = dram.tile((B, HPC, P, KCP, DH), F16, kind="ExternalInput")
            wo_d = dram.tile((P, HPC, D), F16, kind="ExternalInput")
            pout_d = dram.tile((D, TOK), F16, kind="ExternalOutput")
            knT_d = dram.tile((HPC, DH, TOK), F16, kind="ExternalOutput")
            vn_d = dram.tile((TOK, HPC * DH), F16, kind="ExternalOutput")

            with (
                tc.tile_pool(name="const", bufs=1) as const,
                tc.tile_pool(name="persist", bufs=1) as persist,
            ):
                ones_sb = const.tile([P, P], F16)
                nc.any.memset(ones_sb[:], 1.0)

                # PE warmup: ~7us of dummy matmuls during the initial DMA
                # fill (PE is otherwise idle until the first xT chunk lands)
                # so the HAM clock gate reaches 8/8 before real work starts.
                warm_sb = const.tile([P, QB], F16)
                nc.vector.memset(warm_sb[:], 1.0)
                with tc.tile_pool(name="wpsum", bufs=1, space="PSUM") as wpsum:
                    wps = wpsum.tile([P, QB], F32)
                    for w in range(16):
                        nc.tensor.matmul(
                            wps[:], ones_sb[:], warm_sb[:], start=True, stop=True
                        )
                    # consume the result so the warmup isn't dead-code-eliminated
                    nc.vector.tensor_copy(warm_sb[:, 0:P], wps[:, 0:P])

                # persistent intermediates
                # qkT[ct][d, tok]: ct 0/1 = q head0/1, 2/3 = k_new head0/1
                qkT = [
                    persist.tile([P, TOK], F16, tag=f"qkT{ct}", name=f"qkT{ct}")
                    for ct in range(4)
                ]
                # v_new [tok%128, tokchunk, (hlocal d)]
                vn_sb = persist.tile([P, TOK // P, HPC * DH], F16)
                # o^T per (b, j): [d, q]
                o_sb = [
                    [
                        persist.tile([P, L], F16, tag=f"o{b}{j}", name=f"o{b}{j}")
                        for j in range(HPC)
                    ]
                    for b in range(B)
                ]

                # ---- phase-A inputs (latency-critical, sync queue) ----
                # xT resident in SBUF for all of phase A; per-dc tiles keep
                # the first matmuls from waiting on the whole 8MB transfer.
                aload_cm = tc.tile_pool(name="aload", bufs=1)
                aload = aload_cm.__enter__()
                wqk_sb = [
                    aload.tile([P, 4 * P], F16, tag=f"wqk{dc}", name=f"wqk{dc}")
                    for dc in range(DC)
                ]
                xT_sb = [
                    [
                        aload.tile(
                            [P, L], F16, tag=f"xT{h}_{dc}", name=f"xT{h}_{dc}"
                        )
                        for dc in range(DC)
                    ]
                    for h in range(2)
                ]
                # stripe input DMAs across the two DMA-capable engine queues;
                # first token-half first so phase A's opening sweep needs only
                # 6MB (~17us) against its 27us of PE work.
                _qs = [nc.sync, nc.scalar]
                for dc in range(DC):
                    eng = _qs[dc % 2]
                    eng.dma_start(wqk_sb[dc][:], wqk_d[:, dc, :])
                    eng.dma_start(xT_sb[0][dc][:], xp_d[:, dc, 0:L])
                # gate xTb behind most of the first half so it doesn't
                # steal HBM bandwidth from the opening PE-paced sweep
                gate_sb = persist.tile([P, 1], F16)
                nc.gpsimd.tensor_copy(gate_sb[:], xT_sb[0][10][:, 0:1])
                for dc in range(DC):
                    nc.gpsimd.dma_start(xT_sb[1][dc][:], xp_d[:, dc, L : 2 * L])
                wv_sb = aload.tile([P, DC, HPC * DH], F16)
                nc.sync.dma_start(wv_sb[:], wv_d[:])
                bqk_sb = persist.tile([P, 4], F32)
                nc.sync.dma_start(
                    bqk_sb[:], bqk_d[:].rearrange("(ct p) -> p ct", p=P)
                )
                bv_sb = persist.tile([1, HPC * DH], F32)
                nc.sync.dma_start(bv_sb[:], bv_d[None, :])
                bvB_sb = persist.tile([P, HPC * DH], F32)
                nc.gpsimd.partition_broadcast(bvB_sb[:], bv_sb[:])

                # phase B/C inputs staged on the gpsimd queue
                kcT_sb = [
                    [
                        persist.tile(
                            [P, KCP, P], F16, tag=f"kcT{b}{j}", name=f"kcT{b}{j}"
                        )
                        for j in range(HPC)
                    ]
                    for b in range(B)
                ]
                vc_sb = [
                    [
                        persist.tile(
                            [P, KCP, DH], F16, tag=f"vc{b}{j}", name=f"vc{b}{j}"
                        )
                        for j in range(HPC)
                    ]
                    for b in range(B)
                ]
                for b in range(B):
                    for j in range(HPC):
                        nc.gpsimd.dma_start(
                            kcT_sb[b][j][:],
                            kcT_d[b, j].rearrange("d (ko ki) -> d ko ki", ki=P),
                        )
                        nc.gpsimd.dma_start(vc_sb[b][j][:], vc_d[b, j])
                wo_sb = persist.tile([P, HPC, D], F16)
                nc.gpsimd.dma_start(wo_sb[:], wo_d[:])

                # ---- Phase A: qkv projections ----
                # Per token half: 4 col-tiles x 2 token blocks (8 PSUM
                # groups); each arriving 640KB (wqk+xTa per dc) unlocks
                # ~1.7us of PE work while only the first half gates startup.
                with tc.tile_pool(name="apsum", bufs=1, space="PSUM") as apsum:
                    for h in range(2):
                        ps = [
                            [
                                apsum.tile(
                                    [P, QB],
                                    F32,
                                    tag=f"ps{ct}{t}",
                                    bufs=1,
                                    name=f"ps{ct}{t}",
                                )
                                for t in range(2)
                            ]
                            for ct in range(4)
                        ]
                        for dc in range(DC):
                            for ct in range(4):
                                for t in range(2):
                                    nc.tensor.matmul(
                                        ps[ct][t][:],
                                        wqk_sb[dc][:, ct * P : (ct + 1) * P],
                                        xT_sb[h][dc][:, t * QB : (t + 1) * QB],
                                        start=(dc == 0),
                                        stop=(dc == DC - 1),
                                    )
                        for ct in range(4):
                            for t in range(2):
                                nc.scalar.activation(
                                    qkT[ct][
                                        :, h * L + t * QB : h * L + (t + 1) * QB
                                    ],
                                    ps[ct][t][:],
                                    AF.Identity,
                                    bias=bqk_sb[:, ct : ct + 1],
                                )
                    for tc_ in range(TOK // P):  # 16 x 128-token chunks
                        psv = apsum.tile(
                            [P, HPC * DH],
                            F32,
                            tag=f"ps{tc_ % 2}0",
                            bufs=1,
                            name=f"psv{tc_ % 2}",
                        )
                        for dc in range(DC):
                            nc.tensor.matmul(
                                psv[:],
                                xT_sb[tc_ // 8][dc][
                                    :, (tc_ % 8) * P : (tc_ % 8 + 1) * P
                                ],
                                wv_sb[:, dc, :],
                                start=(dc == 0),
                                stop=(dc == DC - 1),
                            )
                        nc.vector.tensor_tensor(
                            vn_sb[:, tc_, :], psv[:], bvB_sb[:], OP.add
                        )

                    # new-key outputs (overlap with phase B)
                    for j in range(HPC):
                        nc.gpsimd.dma_start(knT_d[j], qkT[2 + j][:])
                    nc.gpsimd.dma_start(
                        vn_d[:].rearrange("(tc p) c -> p tc c", p=P), vn_sb[:]
                    )

                aload_cm.__exit__(None, None, None)

                # ---- Phase B: attention per (b, head) ----
                with (
                    tc.tile_pool(name="bpool", bufs=2) as bpool,
                    tc.tile_pool(name="bpsum", bufs=1, space="PSUM") as bpsum,
                ):
                    for b in range(B):
                        for j in range(HPC):
                            for qb in range(NQB):
                                qT = qkT[j][
                                    :, b * L + qb * QB : b * L + (qb + 1) * QB
                                ]
                                A_sb = bpool.tile([P, KC, QB], F16, tag="A")
                                ps_o = bpsum.tile([P, QB], F32, tag="po", bufs=2)
                                ps_sum = bpsum.tile(
                                    [P, QB], F32, tag="psums", bufs=2
                                )

                                def st_mm(pp):
                                    # scores + exp for key chunks 2*pp, 2*pp+1
                                    ps_s = bpsum.tile(
                                        [P, 2, QB], F32, tag="pss", bufs=2
                                    )
                                    for i in range(2):
                                        kk = 2 * pp + i
                                        if kk < KCP:
                                            kT = kcT_sb[b][j][:, kk, :]
                                        else:
                                            kn = kk - KCP
                                            kT = qkT[2 + j][
                                                :,
                                                b * L
                                                + kn * P : b * L
                                                + (kn + 1) * P,
                                            ]
                                        nc.tensor.matmul(
                                            ps_s[:, i, :],
                                            kT,
                                            qT,
                                            start=True,
                                            stop=True,
                                        )
                                    nc.scalar.activation(
                                        A_sb[:, 2 * pp : 2 * pp + 2, :],
                                        ps_s[:],
                                        AF.Exp,
                                        scale=SCALE,
                                    )

                                def sum_pv(kk):
                                    Ak = A_sb[:, kk, :]
                                    nc.tensor.matmul(
                                        ps_sum[:],
                                        ones_sb[:],
                                        Ak,
                                        start=(kk == 0),
                                        stop=(kk == KC - 1),
                                    )
                                    if kk < KCP:
                                        v_chunk = vc_sb[b][j][:, kk, :]
                                    else:
                                        v_chunk = vn_sb[
                                            :,
                                            b * (L // P) + (kk - KCP),
                                            j * DH : (j + 1) * DH,
                                        ]
                                    nc.tensor.matmul(
                                        ps_o[:],
                                        v_chunk,
                                        Ak,
                                        start=(kk == 0),
                                        stop=(kk == KC - 1),
                                    )

                                # software pipeline: scores run one pair ahead
                                st_mm(0)
                                for pp in range(KC // 2):
                                    if pp + 1 < KC // 2:
                                        st_mm(pp + 1)
                                    sum_pv(2 * pp)
                                    sum_pv(2 * pp + 1)

                                recip = bpool.tile([P, QB], F32, tag="recip")
                                nc.vector.reciprocal_approx_fast(recip[:], ps_sum[:])
                                nc.vector.tensor_tensor(
                                    o_sb[b][j][:, qb * QB : (qb + 1) * QB],
                                    ps_o[:],
                                    recip[:],
                                    OP.mult,
                                )

                # ---- Phase C: partial output projection (out^T) ----
                # pout^T[col, tok] = sum_j wo[:, j, col-tile]^T @ o^T[j]
                with (
                    tc.tile_pool(name="cpool", bufs=3) as cpool,
                    tc.tile_pool(name="cpsum", bufs=1, space="PSUM") as cpsum,
                ):
                    NTB = TOK // QB
                    for ct in range(D // P):
                        ps = [
                            cpsum.tile(
                                [P, QB], F32, tag=f"pout{tb}", bufs=2,
                                name=f"pout{tb}",
                            )
                            for tb in range(NTB)
                        ]
                        for j in range(HPC):
                            for tb in range(NTB):
                                b_, qb = divmod(tb, NQB)
                                nc.tensor.matmul(
                                    ps[tb][:],
                                    wo_sb[:, j, ct * P : (ct + 1) * P],
                                    o_sb[b_][j][:, qb * QB : (qb + 1) * QB],
                                    start=(j == 0),
                                    stop=(j == HPC - 1),
                                )
                        for hh in range(2):
                            out_t = cpool.tile(
                                [P, L], F16, tag=f"outsb{hh}", name=f"outsb{hh}"
                            )
                            for t2 in range(2):
                                tb = hh * 2 + t2
                                if tb % 2 == 0:
                                    nc.vector.tensor_copy(
                                        out_t[:, t2 * QB : (t2 + 1) * QB],
                                        ps[tb][:],
                                    )
                                else:
                                    nc.scalar.copy(
                                        out_t[:, t2 * QB : (t2 + 1) * QB],
                                        ps[tb][:],
                                    )
                            nc.sync.dma_start(
                                pout_d[ct * P : (ct + 1) * P, hh * L : (hh + 1) * L],
                                out_t[:],
                            )
    nc.compile()
    names = dict(
        xT=xp_d.name,
        wqk=wqk_d.name,
        wv=wv_d.name,
        bqk=bqk_d.name,
        bv=bv_d.name,
        kcT=kcT_d.name,
        vc=vc_d.name,
        wo=wo_d.name,
        pout=pout_d.name,
        knT=knT_d.name,
        vn=vn_d.name,
    )
    return nc, names


def kernel(x, k_active_cache, v_active_cache, Wqkv, bqkv, Wout, bout):
    global LAST_RESULT
    x = np.asarray(x, dtype=np.float32)
    kc = np.ascontiguousarray(np.asarray(k_active_cache, dtype=np.float32))
    vc = np.ascontiguousarray(np.asarray(v_active_cache, dtype=np.float32))
    Wqkv = np.asarray(Wqkv, dtype=np.float32)
    bqkv = np.asarray(bqkv, dtype=np.float32)
    Wout = np.asarray(Wout, dtype=np.float32)
    bout = np.asarray(bout, dtype=np.float32)

    if "nc" not in _CACHE:
        _CACHE["nc"] = _build()
    nc, nm = _CACHE["nc"]

    xp = np.ascontiguousarray(
        x.reshape(TOK, DC, P).transpose(2, 1, 0).astype(np.float16)
    )
    Wq = Wqkv[:, 0:D]
    Wk = Wqkv[:, D : 2 * D]
    Wv = Wqkv[:, 2 * D : 3 * D]
    bq, bk, bv = bqkv[0:D], bqkv[D : 2 * D], bqkv[2 * D : 3 * D]
    # [b, h, d, keys] cache, transposed per head
    kcT = np.ascontiguousarray(kc.transpose(0, 1, 3, 2).astype(np.float16))
    vc16 = vc.astype(np.float16)

    in_maps = []
    for c in range(N_CORES):
        cs = slice(c * HPC * DH, (c + 1) * HPC * DH)
        in_maps.append(
            {
                nm["xT"]: xp,
                nm["wqk"]: np.ascontiguousarray(
                    np.concatenate([Wq[:, cs], Wk[:, cs]], axis=1)
                    .astype(np.float16)
                    .reshape(DC, P, 4 * P)
                    .transpose(1, 0, 2)
                ),
                nm["wv"]: np.ascontiguousarray(
                    Wv[:, cs]
                    .astype(np.float16)
                    .reshape(DC, P, HPC * DH)
                    .transpose(1, 0, 2)
                ),
                nm["bqk"]: np.ascontiguousarray(np.concatenate([bq[cs], bk[cs]])),
                nm["bv"]: np.ascontiguousarray(bv[cs]),
                nm["kcT"]: np.ascontiguousarray(kcT[:, c * HPC : (c + 1) * HPC]),
                nm["vc"]: np.ascontiguousarray(
                    vc16[:, c * HPC : (c + 1) * HPC]
                    .reshape(B, HPC, KCP, P, DH)
                    .transpose(0, 1, 3, 2, 4)
                ),
                nm["wo"]: np.ascontiguousarray(
                    Wout[cs, :]
                    .astype(np.float16)
                    .reshape(HPC, P, D)
                    .transpose(1, 0, 2)
                ),
            }
        )

    last_exc = None
    for attempt in range(3):
        try:
            LAST_RESULT = run_bass_kernel_spmd(
                nc, in_maps, core_ids=list(range(N_CORES))
            )
            break
        except Exception as e:  # transient device errors (NRT_EXEC_UNIT_...)
            last_exc = e
    else:
        raise last_exc
    results = LAST_RESULT.results

    # ---- host assembly ----
    poutT = np.zeros((D, TOK), dtype=np.float32)
    for c in range(N_CORES):
        poutT += results[c][nm["pout"]].astype(np.float32)
    out = poutT.T + bout
    out = out.reshape(B, L, D)

    k_full = np.empty((B, H, LP + L, DH), dtype=np.float32)
    v_full = np.empty((B, H, LP + L, DH), dtype=np.float32)
    k_full[:, :, :LP] = kc
    v_full[:, :, :LP] = vc
    for c in range(N_CORES):
        knT = results[c][nm["knT"]].astype(np.float32)  # [HPC, DH, TOK]
        vn = results[c][nm["vn"]].astype(np.float32).reshape(B, L, HPC, DH)
        for j in range(HPC):
            h = c * HPC + j
            for b in range(B):
                k_full[b, h, LP:] = knT[j, :, b * L : (b + 1) * L].T
                v_full[b, h, LP:] = vn[b, :, j, :]
    return out, k_full, v_full


# revision 27
# speedup vs baseline: 1.0113x; 1.0113x over previous
"""Tensor-parallel attention block for 8 TRN2 NeuronCores.

Reference computation (per nn_Attention_63359357550974):
    qkv = x @ Wqkv + bqkv ; split into q, k_new, v_new per head
    k = concat(k_cache, k_new); v = concat(v_cache, v_new)
    o = softmax(q @ k^T / sqrt(d)) @ v
    out = o @ Wout + bout ; returns (out, k, v)

Sharding: 16 heads / 8 cores = 2 heads per core (tensor parallel).  Each
core computes its heads' QKV columns from the full x, runs attention for
its 4 (batch, head) pairs, and produces a partial output projection
(contracting only its 256 o-columns of Wout).  The host sums the 8
partial projections and adds bout; k/v outputs are assembled host-side
from the cache plus each core's new-key tensors.  No device collectives.

Device layout choices (matmul inputs in fp16: 1 cycle/row PE streaming —
fp32/fp32r stream at 2 cycles/row — while keeping ~1e-3 accuracy):
  - x is host-transposed/cast to xT [dim, tok] fp16 so qkv^T needs no
    on-device transposes; q^T / k_new^T come out directly as [d, tok].
  - scores are computed transposed: S^T[keys, q] = kT-chunk^T @ qT, so
    exp(scale * S^T) -> A^T is directly the PV moving operand.  No
    max-subtraction (|scores*scale| stays < ~6 for this data regime).
  - softmax denominators via ones[128,128]-stationary matmuls: every
    PSUM partition receives the column sum, so the reciprocal runs on
    all 128 lanes and no partition broadcast is needed.
  - o^T = V^T @ A^T accumulated over 16 key chunks, multiplied by the
    replicated reciprocal.
  - partial out[tok, col] = o^T-stationary @ Wout-moving, DMA'd out.
"""

import numpy as np

import concourse.tile as tile
from concourse import bacc, mybir
from concourse.bass_utils import run_bass_kernel_spmd

P = 128
B = 2
L = 1024  # new tokens per batch
LP = 1024  # cached keys per batch
D = 2048
H = 16
DH = 128  # head dim
N_CORES = 8
HPC = H // N_CORES  # heads per core = 2
TOK = B * L  # 2048
DC = D // P  # 16 dim chunks
KC = (LP + L) // P  # 16 key chunks per (b, h)
KCP = LP // P  # 8 cached key chunks
QB = 512  # q block (moving free dim)
NQB = L // QB  # 2

F32 = mybir.dt.float32
F16 = mybir.dt.float16
SCALE = 1.0 / float(np.sqrt(DH))

LAST_RESULT = None
_CACHE = {}


def _build():
    nc = bacc.Bacc(None, target_bir_lowering=False)
    AF = mybir.ActivationFunctionType
    OP = mybir.AluOpType
    with tile.TileContext(nc) as tc:
        with tc.tile_pool(name="dram", bufs=1, space="DRAM") as dram:
            xp_d = dram.tile((P, DC, TOK), F16, kind="ExternalInput")
            wqk_d = dram.tile((P, DC, 4 * P), F16, kind="ExternalInput")
            wv_d = dram.tile((P, DC, HPC * DH), F16, kind="ExternalInput")
            bqk_d = dram.tile((4 * P,), F32, kind="ExternalInput")
            bv_d = dram.tile((HPC * DH,), F32, kind="ExternalInput")
            kcT_d = dram.tile((B, HPC, DH, LP), F16, kind="ExternalInput")
            vc_d = dram.tile((B, HPC, P, KCP, DH), F16, kind="ExternalInput")
            wo_d = dram.tile((P, HPC, D), F16, kind="ExternalInput")
            pout_d = dram.tile((D, TOK), F16, kind="ExternalOutput")
            knT_d = dram.tile((HPC, DH, TOK), F16, kind="ExternalOutput")
            vn_d = dram.tile((TOK, HPC * DH), F16, kind="ExternalOutput")

            with (
                tc.tile_pool(name="const", bufs=1) as const,
                tc.tile_pool(name="persist", bufs=1) as persist,
            ):
                ones_sb = const.tile([P, P], F16)
                nc.any.memset(ones_sb[:], 1.0)

                # PE warmup: ~7us of dummy matmuls during the initial DMA
                # fill (PE is otherwise idle until the first xT chunk lands)
                # so the HAM clock gate reaches 8/8 before real work starts.
                warm_sb = const.tile([P, QB], F16)
                nc.vector.memset(warm_sb[:], 1.0)
                with tc.tile_pool(name="wpsum", bufs=1, space="PSUM") as wpsum:
                    wps = wpsum.tile([P, QB], F32)
                    for w in range(16):
                        nc.tensor.matmul(
                            wps[:], ones_sb[:], warm_sb[:], start=True, stop=True
                        )
                    # consume the result so the warmup isn't dead-code-eliminated
                    nc.vector.tensor_copy(warm_sb[:, 0:P], wps[:, 0:P])

                # persistent intermediates
                # qkT[ct][d, tok]: ct 0/1 = q head0/1, 2/3 = k_new head0/1
                qkT = [
                    persist.tile([P, TOK], F16, tag=f"qkT{ct}", name=f"qkT{ct}")
                    for ct in range(4)
                ]
                # v_new [tok%128, tokchunk, (hlocal d)]
                vn_sb = persist.tile([P, TOK // P, HPC * DH], F16)
                # o^T per (b, j): [d, q]
                o_sb = [
                    [
                        persist.tile([P, L], F16, tag=f"o{b}{j}", name=f"o{b}{j}")
                        for j in range(HPC)
                    ]
                    for b in range(B)
                ]

                # ---- phase-A inputs (latency-critical, sync queue) ----
                # xT resident in SBUF for all of phase A; per-dc tiles keep
                # the first matmuls from waiting on the whole 8MB transfer.
                aload_cm = tc.tile_pool(name="aload", bufs=1)
                aload = aload_cm.__enter__()
                wqk_sb = [
                    aload.tile([P, 4 * P], F16, tag=f"wqk{dc}", name=f"wqk{dc}")
                    for dc in range(DC)
                ]
                xT_sb = [
                    [
                        aload.tile(
                            [P, L], F16, tag=f"xT{h}_{dc}", name=f"xT{h}_{dc}"
                        )
                        for dc in range(DC)
                    ]
                    for h in range(2)
                ]
                # stripe input DMAs across the two DMA-capable engine queues;
                # first token-half first so phase A's opening sweep needs only
                # 6MB (~17us) against its 27us of PE work.
                _qs = [nc.sync, nc.scalar]
                for dc in range(DC):
                    eng = _qs[dc % 2]
                    eng.dma_start(wqk_sb[dc][:], wqk_d[:, dc, :])
                    eng.dma_start(xT_sb[0][dc][:], xp_d[:, dc, 0:L])
                for dc in range(DC):
                    nc.gpsimd.dma_start(xT_sb[1][dc][:], xp_d[:, dc, L : 2 * L])
                wv_sb = aload.tile([P, DC, HPC * DH], F16)
                nc.sync.dma_start(wv_sb[:], wv_d[:])
                bqk_sb = persist.tile([P, 4], F32)
                nc.sync.dma_start(
                    bqk_sb[:], bqk_d[:].rearrange("(ct p) -> p ct", p=P)
                )
                bv_sb = persist.tile([1, HPC * DH], F32)
                nc.sync.dma_start(bv_sb[:], bv_d[None, :])
                bvB_sb = persist.tile([P, HPC * DH], F32)
                nc.gpsimd.partition_broadcast(bvB_sb[:], bv_sb[:])

                # phase B/C inputs staged on the gpsimd queue
                kcT_sb = [
                    [
                        persist.tile(
                            [P, KCP, P], F16, tag=f"kcT{b}{j}", name=f"kcT{b}{j}"
                        )
                        for j in range(HPC)
                    ]
                    for b in range(B)
                ]
                vc_sb = [
                    [
                        persist.tile(
                            [P, KCP, DH], F16, tag=f"vc{b}{j}", name=f"vc{b}{j}"
                        )
                        for j in range(HPC)
                    ]
                    for b in range(B)
                ]
                for b in range(B):
                    for j in range(HPC):
                        nc.gpsimd.dma_start(
                            kcT_sb[b][j][:],
                            kcT_d[b, j].rearrange("d (ko ki) -> d ko ki", ki=P),
                        )
                        nc.gpsimd.dma_start(vc_sb[b][j][:], vc_d[b, j])
                wo_sb = persist.tile([P, HPC, D], F16)
                nc.gpsimd.dma_start(wo_sb[:], wo_d[:])

                # ---- Phase A: qkv projections ----
                # Per token half: 4 col-tiles x 2 token blocks (8 PSUM
                # groups); each arriving 640KB (wqk+xTa per dc) unlocks
                # ~1.7us of PE work while only the first half gates startup.
                with tc.tile_pool(name="apsum", bufs=1, space="PSUM") as apsum:
                    for h in range(2):
                        ps = [
                            [
                                apsum.tile(
                                    [P, QB],
                                    F32,
                                    tag=f"ps{ct}{t}",
                                    bufs=1,
                                    name=f"ps{ct}{t}",
                                )
                                for t in range(2)
                            ]
                            for ct in range(4)
                        ]
                        for dc in range(DC):
                            for ct in range(4):
                                for t in range(2):
                                    nc.tensor.matmul(
                                        ps[ct][t][:],
                                        wqk_sb[dc][:, ct * P : (ct + 1) * P],
                                        xT_sb[h][dc][:, t * QB : (t + 1) * QB],
                                        start=(dc == 0),
                                        stop=(dc == DC - 1),
                                    )
                        for ct in range(4):
                            for t in range(2):
                                nc.scalar.activation(
                                    qkT[ct][
                                        :, h * L + t * QB : h * L + (t + 1) * QB
                                    ],
                                    ps[ct][t][:],
                                    AF.Identity,
                                    bias=bqk_sb[:, ct : ct + 1],
                                )
                    for tc_ in range(TOK // P):  # 16 x 128-token chunks
                        psv = apsum.tile(
                            [P, HPC * DH],
                            F32,
                            tag=f"ps{tc_ % 2}0",
                            bufs=1,
                            name=f"psv{tc_ % 2}",
                        )
                        for dc in range(DC):
                            nc.tensor.matmul(
                                psv[:],
                                xT_sb[tc_ // 8][dc][
                                    :, (tc_ % 8) * P : (tc_ % 8 + 1) * P
                                ],
                                wv_sb[:, dc, :],
                                start=(dc == 0),
                                stop=(dc == DC - 1),
                            )
                        nc.vector.tensor_tensor(
                            vn_sb[:, tc_, :], psv[:], bvB_sb[:], OP.add
                        )

                    # new-key outputs (overlap with phase B)
                    for j in range(HPC):
                        nc.gpsimd.dma_start(knT_d[j], qkT[2 + j][:])
                    nc.gpsimd.dma_start(
                        vn_d[:].rearrange("(tc p) c -> p tc c", p=P), vn_sb[:]
                    )

                aload_cm.__exit__(None, None, None)

                # ---- Phase B: attention per (b, head) ----
                with (
                    tc.tile_pool(name="bpool", bufs=2) as bpool,
                    tc.tile_pool(name="bpsum", bufs=1, space="PSUM") as bpsum,
                ):
                    for b in range(B):
                        for j in range(HPC):
                            for qb in range(NQB):
                                qT = qkT[j][
                                    :, b * L + qb * QB : b * L + (qb + 1) * QB
                                ]
                                A_sb = bpool.tile([P, KC, QB], F16, tag="A")
                                ps_o = bpsum.tile([P, QB], F32, tag="po", bufs=2)
                                ps_sum = bpsum.tile(
                                    [P, QB], F32, tag="psums", bufs=2
                                )

                                def st_mm(pp):
                                    # scores + exp for key chunks 2*pp, 2*pp+1
                                    ps_s = bpsum.tile(
                                        [P, 2, QB], F32, tag="pss", bufs=2
                                    )
                                    for i in range(2):
                                        kk = 2 * pp + i
                                        if kk < KCP:
                                            kT = kcT_sb[b][j][:, kk, :]
                                        else:
                                            kn = kk - KCP
                                            kT = qkT[2 + j][
                                                :,
                                                b * L
                                                + kn * P : b * L
                                                + (kn + 1) * P,
                                            ]
                                        nc.tensor.matmul(
                                            ps_s[:, i, :],
                                            kT,
                                            qT,
                                            start=True,
                                            stop=True,
                                        )
                                    nc.scalar.activation(
                                        A_sb[:, 2 * pp : 2 * pp + 2, :],
                                        ps_s[:],
                                        AF.Exp,
                                        scale=SCALE,
                                    )

                                def sum_pv(kk):
                                    Ak = A_sb[:, kk, :]
                                    nc.tensor.matmul(
                                        ps_sum[:],
                                        ones_sb[:],
                                        Ak,
                                        start=(kk == 0),
                                        stop=(kk == KC - 1),
                                    )
                                    if kk < KCP:
                                        v_chunk = vc_sb[b][j][:, kk, :]
                                    else:
                                        v_chunk = vn_sb[
                                            :,
                                            b * (L // P) + (kk - KCP),
                                            j * DH : (j + 1) * DH,
                                        ]
                                    nc.tensor.matmul(
                                        ps_o[:],
                                        v_chunk,
                                        Ak,
                                        start=(kk == 0),
                                        stop=(kk == KC - 1),
                                    )

                                # software pipeline: scores run one pair ahead
                                st_mm(0)
                                for pp in range(KC // 2):
                                    if pp + 1 < KC // 2:
                                        st_mm(pp + 1)
                                    sum_pv(2 * pp)
                                    sum_pv(2 * pp + 1)

                                recip = bpool.tile([P, QB], F32, tag="recip")
                                nc.vector.reciprocal_approx_fast(recip[:], ps_sum[:])
                                nc.vector.tensor_tensor(
                                    o_sb[b][j][:, qb * QB : (qb + 1) * QB],
                                    ps_o[:],
                                    recip[:],
                                    OP.mult,
                                )

                # ---- Phase C: partial output projection (out^T) ----
                # pout^T[col, tok] = sum_j wo[:, j, col-tile]^T @ o^T[j]
                with (
                    tc.tile_pool(name="cpool", bufs=3) as cpool,
                    tc.tile_pool(name="cpsum", bufs=1, space="PSUM") as cpsum,
                ):
                    NTB = TOK // QB
                    for ct in range(D // P):
                        ps = [
                            cpsum.tile(
                                [P, QB], F32, tag=f"pout{tb}", bufs=2,
                                name=f"pout{tb}",
                            )
                            for tb in range(NTB)
                        ]
                        for j in range(HPC):
                            for tb in range(NTB):
                                b_, qb = divmod(tb, NQB)
                                nc.tensor.matmul(
                                    ps[tb][:],
                                    wo_sb[:, j, ct * P : (ct + 1) * P],
                                    o_sb[b_][j][:, qb * QB : (qb + 1) * QB],
                                    start=(j == 0),
                                    stop=(j == HPC - 1),
                                )
                        for hh in range(2):
                            out_t = cpool.tile(
                                [P, L], F16, tag=f"outsb{hh}", name=f"outsb{hh}"
                            )
                            for t2 in range(2):
                                tb = hh * 2 + t2
                                if tb % 2 == 0:
                                    nc.vector.tensor_copy(
                                        out_t[:, t2 * QB : (t2 + 1) * QB],
                                        ps[tb][:],
                                    )
                                else:
                                    nc.scalar.copy(
                                        out_t[:, t2 * QB : (t2 + 1) * QB],
                                        ps[tb][:],
                                    )
                            nc.sync.dma_start(
                                pout_d[ct * P : (ct + 1) * P, hh * L : (hh + 1) * L],
                                out_t[:],
                            )
    nc.compile()
    names = dict(
        xT=xp_d.name,
        wqk=wqk_d.name,
        wv=wv_d.name,
        bqk=bqk_d.name,
        bv=bv_d.name,
        kcT=kcT_d.name,
        vc=vc_d.name,
        wo=wo_d.name,
        pout=pout_d.name,
        knT=knT_d.name,
        vn=vn_d.name,
    )
    return nc, names


def kernel(x, k_active_cache, v_active_cache, Wqkv, bqkv, Wout, bout):
    global LAST_RESULT
    x = np.asarray(x, dtype=np.float32)
    kc = np.ascontiguousarray(np.asarray(k_active_cache, dtype=np.float32))
    vc = np.ascontiguousarray(np.asarray(v_active_cache, dtype=np.float32))
    Wqkv = np.asarray(Wqkv, dtype=np.float32)
    bqkv = np.asarray(bqkv, dtype=np.float32)
    Wout = np.asarray(Wout, dtype=np.float32)
    bout = np.asarray(bout, dtype=np.float32)

    if "nc" not in _CACHE:
        _CACHE["nc"] = _build()
    nc, nm = _CACHE["nc"]

    xp = np.ascontiguousarray(
        x.reshape(TOK, DC, P).transpose(2, 1, 0).astype(np.float16)
    )
    Wq = Wqkv[:, 0:D]
    Wk = Wqkv[:, D : 2 * D]
    Wv = Wqkv[:, 2 * D : 3 * D]
    bq, bk, bv = bqkv[0:D], bqkv[D : 2 * D], bqkv[2 * D : 3 * D]
    # [b, h, d, keys] cache, transposed per head
    kcT = np.ascontiguousarray(kc.transpose(0, 1, 3, 2).astype(np.float16))
    vc16 = vc.astype(np.float16)

    in_maps = []
    for c in range(N_CORES):
        cs = slice(c * HPC * DH, (c + 1) * HPC * DH)
        in_maps.append(
            {
                nm["xT"]: xp,
                nm["wqk"]: np.ascontiguousarray(
                    np.concatenate([Wq[:, cs], Wk[:, cs]], axis=1)
                    .astype(np.float16)
                    .reshape(DC, P, 4 * P)
                    .transpose(1, 0, 2)
                ),
                nm["wv"]: np.ascontiguousarray(
                    Wv[:, cs]
                    .astype(np.float16)
                    .reshape(DC, P, HPC * DH)
                    .transpose(1, 0, 2)
                ),
                nm["bqk"]: np.ascontiguousarray(np.concatenate([bq[cs], bk[cs]])),
                nm["bv"]: np.ascontiguousarray(bv[cs]),
                nm["kcT"]: np.ascontiguousarray(kcT[:, c * HPC : (c + 1) * HPC]),
                nm["vc"]: np.ascontiguousarray(
                    vc16[:, c * HPC : (c + 1) * HPC]
                    .reshape(B, HPC, KCP, P, DH)
                    .transpose(0, 1, 3, 2, 4)
                ),
                nm["wo"]: np.ascontiguousarray(
                    Wout[cs, :]
                    .astype(np.float16)
                    .reshape(HPC, P, D)
                    .transpose(1, 0, 2)
                ),
            }
        )

    last_exc = None
    for attempt in range(3):
        try:
            LAST_RESULT = run_bass_kernel_spmd(
                nc, in_maps, core_ids=list(range(N_CORES))
            )
            break
        except Exception as e:  # transient device errors (NRT_EXEC_UNIT_...)
            last_exc = e
    else:
        raise last_exc
    results = LAST_RESULT.results

    # ---- host assembly ----
    poutT = np.zeros((D, TOK), dtype=np.float32)
    for c in range(N_CORES):
        poutT += results[c][nm["pout"]].astype(np.float32)
    out = poutT.T + bout
    out = out.reshape(B, L, D)

    k_full = np.empty((B, H, LP + L, DH), dtype=np.float32)
    v_full = np.empty((B, H, LP + L, DH), dtype=np.float32)
    k_full[:, :, :LP] = kc
    v_full[:, :, :LP] = vc
    for c in range(N_CORES):
        knT = results[c][nm["knT"]].astype(np.float32)  # [HPC, DH, TOK]
        vn = results[c][nm["vn"]].astype(np.float32).reshape(B, L, HPC, DH)
        for j in range(HPC):
            h = c * HPC + j
            for b in range(B):
                k_full[b, h, LP:] = knT[j, :, b * L : (b + 1) * L].T
                v_full[b, h, LP:] = vn[b, :, j, :]
    return out, k_full, v_full
